# revision 1
# baseline (speedup 1.0000x reference)
"""Trainium2 Bass kernel for nn_GN_89266600280080.

Computes, for output[B,O], input[B,D], weights[O]:
    dl_dW = (1/B) * (output * weights)^T @ input        # [O, D]
    gw    = sqrt(sum(dl_dW^2, axis=1))                  # [O]

Strategy (8 NeuronCores, data-parallel over batch):
  - host: shard output/input on batch, pre-pack per-core slices into
    [128, n_blk, *] partition-major layout and quantize to fp8e4m3
    (norm over D=1024 averages quantization noise: ~3e-3 rel err vs
    2e-2 tolerance). Weight folding deferred to the final host scalar.
  - device: stacked partials = output_loc^T @ input_loc via fp8
    matmuls, 4-way column-tiled (tile_position=(0,32g)): 4 batch
    blocks stream concurrently through disjoint 32-col strips of the
    PE array. Input DMA round-robins over the 3 DMA rings (sync-HWDGE,
    scalar-HWDGE, gpsimd-SWDGE) with coalesced 4KB/partition
    descriptors to overlap ring throughputs.
  - each core emits its stacked [128, D] partial (bf16); host sums
    the 4 column groups + 8 cores and finishes the (tiny) norm.
"""

import sys
import numpy as np
import ml_dtypes

for _p in ("/opt/trn_rl_repo", "/root/.axon_site/_ro/trn_rl_repo"):
    if _p not in sys.path:
        sys.path.insert(0, _p)

B, O, D = 32768, 32, 1024
N_CORES = 8
B_LOC = B // N_CORES   # 4096
P = 128                # partitions
NBLK = B_LOC // P      # 32 batch blocks of 128 rows per core
NH = 2                 # D split into NH chunks of 512 for psum banks
ND = D // NH           # 512
CH = 4                 # batch blocks per DMA chunk
NGRP = 4               # concurrent PE column groups
NP8 = ml_dtypes.float8_e4m3


SCHEDULE = (2, 2, 4, 8, 8, 4, 2, 1, 1)   # blocks per DMA chunk (sums to NBLK)


def build(n_cores=N_CORES, schedule=SCHEDULE, nrings=1):
    """Build + compile the per-core Bass program. Returns the Bacc object."""
    import concourse.bacc as bacc
    import concourse.tile as tile
    import concourse.mybir as mybir

    f32 = mybir.dt.float32
    bf16 = mybir.dt.bfloat16
    f8 = mybir.dt.float8e4
    nsteps = NBLK // NGRP  # accumulation steps per column group
    assert sum(schedule) == NBLK

    nc = bacc.Bacc("TRN2", target_bir_lowering=False, debug=False,
                   num_devices=n_cores)

    # host pre-packed layouts (partition-major, contiguous per partition)
    in_d = nc.dram_tensor("input", [P, NBLK, D], f8, kind="ExternalInput")
    out_d = nc.dram_tensor("output", [P, NBLK, O], f8, kind="ExternalInput")
    part_d = nc.dram_tensor("part", [P, D], bf16, kind="ExternalOutput")

    in_2d = in_d.ap().rearrange("p n d -> p (n d)")

    with tile.TileContext(nc) as tc:
        with (
            tc.tile_pool(name="wout", bufs=1) as wout_pool,
            tc.tile_pool(name="rhs", bufs=len(schedule)) as rhs_pool,
            tc.tile_pool(name="ps", bufs=2, space="PSUM") as psum_pool,
            tc.tile_pool(name="misc", bufs=1) as misc,
        ):
            # stationary operand: all local w_out rows, [128, 32, 32] fp8
            wout = wout_pool.tile([P, NBLK, O], f8)
            nc.scalar.dma_start(wout[:], out_d.ap())

            dma_engines = [nc.sync, nc.scalar][:nrings]

            # stacked partials: group g accumulates into partitions 32g..32g+31
            # separate psum tiles per D-half so each half's cast only waits
            # on that half's matmuls
            psums = [psum_pool.tile([P, ND], f32, name=f"psum{h}")
                     for h in range(NH)]
            blk0 = 0
            for c, ch in enumerate(schedule):
                rhs = rhs_pool.tile([P, ch, D], f8)
                # coalesced 2D AP -> ch KB contiguous per partition line
                dma_engines[c % nrings].dma_start(
                    rhs[:].rearrange("p n d -> p (n d)"),
                    in_2d[:, blk0 * D:(blk0 + ch) * D])
                for h in range(NH):
                    for j in range(ch):
                        blk = blk0 + j
                        g, t = blk % NGRP, blk // NGRP
                        nc.tensor.matmul(
                            psums[h][O * g:O * (g + 1), :],
                            wout[:, blk, :],
                            rhs[:, j, h * ND:(h + 1) * ND],
                            start=(t == 0),
                            stop=(t == nsteps - 1),
                            tile_position=(0, O * g),
                        )
                blk0 += ch

            # cast stacked psum halves to bf16 on DVE; ship each half on its
            # own HWDGE ring as soon as it's ready. Host sums the 4 groups +
            # 8 cores and finishes the norm.
            stacked_bf = misc.tile([P, D], bf16)
            for h in range(NH):
                nc.vector.tensor_copy(
                    stacked_bf[:, h * ND:(h + 1) * ND], psums[h][:])
                [nc.sync, nc.scalar][h % 2].dma_start(
                    part_d.ap()[:, h * ND:(h + 1) * ND],
                    stacked_bf[:, h * ND:(h + 1) * ND])

    nc.compile()
    return nc


_CACHE = {}


def _get_nc():
    if "nc" not in _CACHE:
        _CACHE["nc"] = build()
    return _CACHE["nc"]


def _pack(arr, ncols):
    """[B_LOC, ncols] fp32 -> [128, NBLK, ncols] fp8, partition-major."""
    return np.ascontiguousarray(
        arr.reshape(NBLK, P, ncols).transpose(1, 0, 2)).astype(NP8)


def prep_in_maps(inputs):
    output = np.asarray(inputs["output"], dtype=np.float32)
    input = np.asarray(inputs["input"], dtype=np.float32)
    return [
        {
            "output": _pack(output[c * B_LOC:(c + 1) * B_LOC], O),
            "input": _pack(input[c * B_LOC:(c + 1) * B_LOC], D),
        }
        for c in range(N_CORES)
    ]


def kernel(output, input, weights):
    from concourse.bass_utils import run_bass_kernel_spmd

    weights = np.asarray(weights, dtype=np.float32)
    nc = _get_nc()
    in_maps = prep_in_maps({"output": output, "input": input})
    res = run_bass_kernel_spmd(nc, in_maps, list(range(N_CORES)))
    # host finish: sum 4 col-groups + 8 cores, then the tiny norm
    M = np.zeros((O, D), dtype=np.float64)
    for r in res.results:
        part = np.asarray(r["part"]).astype(np.float64)       # [128, D]
        M += part.reshape(NGRP, O, D).sum(axis=0)
    ss = (M * M).sum(axis=1)
    gw = np.sqrt(ss) * (weights.astype(np.float64) / B)
    return gw.astype(np.float32)



# revision 2
# speedup vs baseline: 1.0002x; 1.0002x over previous
"""Trainium2 Bass kernel for nn_GN_89266600280080.

Computes, for output[B,O], input[B,D], weights[O]:
    dl_dW = (1/B) * (output * weights)^T @ input        # [O, D]
    gw    = sqrt(sum(dl_dW^2, axis=1))                  # [O]

Strategy (8 NeuronCores, data-parallel over batch):
  - host: shard output/input on batch, pre-pack per-core slices into
    [128, n_blk, *] partition-major layout and quantize to fp8e4m3
    (norm over D=1024 averages quantization noise: ~3e-3 rel err vs
    2e-2 tolerance). Weight folding deferred to the final host scalar.
  - device: stacked partials = output_loc^T @ input_loc via fp8
    matmuls, 4-way column-tiled (tile_position=(0,32g)): 4 batch
    blocks stream concurrently through disjoint 32-col strips of the
    PE array.
  - input is staged per-chunk as its own contiguous DRAM tensor
    (descriptor p = base + p*chunk_bytes), so each chunk's 128 DMA
    descriptors sweep one contiguous DRAM span: measured ~390-410 GB/s
    on the 8KB-line chunks vs ~320 for the partition-strided layout.
  - schedule front/tail chunks are small (quick first matmul, short
    last-chunk drain); middle chunks are 4-8 blocks for big
    descriptors. All input DMA on the sync HWDGE ring (two rings
    measured slower: interleaved descriptor streams thrash HBM).
  - tail: psum half 0 casts on DVE -> sync ring, half 1 casts on the
    Scalar (Act) engine -> scalar ring, so the two casts and the two
    128KB output DMAs overlap.
  - each core emits its stacked [128, D] partial (bf16); host sums
    the 4 column groups + 8 cores and finishes the (tiny) norm.
"""

import sys
import numpy as np
import ml_dtypes

for _p in ("/opt/trn_rl_repo", "/root/.axon_site/_ro/trn_rl_repo"):
    if _p not in sys.path:
        sys.path.insert(0, _p)

B, O, D = 32768, 32, 1024
N_CORES = 8
B_LOC = B // N_CORES   # 4096
P = 128                # partitions
NBLK = B_LOC // P      # 32 batch blocks of 128 rows per core
NH = 2                 # D split into NH chunks of 512 for psum banks
ND = D // NH           # 512
NGRP = 4               # concurrent PE column groups
NP8 = ml_dtypes.float8_e4m3

SCHEDULE = (2, 2, 4, 8, 8, 4, 2, 1, 1)   # blocks per DMA chunk (sums to NBLK)


def build(n_cores=N_CORES, schedule=SCHEDULE, dual_tail=True):
    """Build + compile the per-core Bass program. Returns the Bacc object."""
    import concourse.bacc as bacc
    import concourse.tile as tile
    import concourse.mybir as mybir

    f32 = mybir.dt.float32
    bf16 = mybir.dt.bfloat16
    f8 = mybir.dt.float8e4
    nsteps = NBLK // NGRP  # accumulation steps per column group
    assert sum(schedule) == NBLK

    nc = bacc.Bacc("TRN2", target_bir_lowering=False, debug=False,
                   num_devices=n_cores)

    # host pre-packed layouts: one contiguous DRAM tensor per input chunk
    ins = [nc.dram_tensor(f"in{c}", [P, ch * D], f8, kind="ExternalInput")
           for c, ch in enumerate(schedule)]
    out_d = nc.dram_tensor("output", [P, NBLK, O], f8, kind="ExternalInput")
    part_d = nc.dram_tensor("part", [P, D], bf16, kind="ExternalOutput")

    with tile.TileContext(nc) as tc:
        with (
            tc.tile_pool(name="wout", bufs=1) as wout_pool,
            tc.tile_pool(name="rhs", bufs=len(schedule)) as rhs_pool,
            tc.tile_pool(name="ps", bufs=2, space="PSUM") as psum_pool,
            tc.tile_pool(name="misc", bufs=1) as misc,
        ):
            # stationary operand: all local w_out rows, [128, 32, 32] fp8
            wout = wout_pool.tile([P, NBLK, O], f8)
            nc.scalar.dma_start(wout[:], out_d.ap())

            # stacked partials: group g accumulates into partitions 32g..32g+31
            # separate psum tiles per D-half so each half's cast only waits
            # on that half's matmuls
            psums = [psum_pool.tile([P, ND], f32, name=f"psum{h}")
                     for h in range(NH)]
            blk0 = 0
            for c, ch in enumerate(schedule):
                rhs = rhs_pool.tile([P, ch, D], f8)
                nc.sync.dma_start(rhs[:].rearrange("p n d -> p (n d)"),
                                  ins[c].ap())
                for h in range(NH):
                    for j in range(ch):
                        blk = blk0 + j
                        g, t = blk % NGRP, blk // NGRP
                        nc.tensor.matmul(
                            psums[h][O * g:O * (g + 1), :],
                            wout[:, blk, :],
                            rhs[:, j, h * ND:(h + 1) * ND],
                            start=(t == 0),
                            stop=(t == nsteps - 1),
                            tile_position=(0, O * g),
                        )
                blk0 += ch

            # tail: the two psum halves cast on different engines and ship
            # on different HWDGE rings so they overlap
            stacked = misc.tile([P, D], bf16)
            if dual_tail:
                nc.vector.tensor_copy(stacked[:, 0:ND], psums[0][:])
                nc.sync.dma_start(part_d.ap()[:, 0:ND], stacked[:, 0:ND])
                nc.scalar.copy(stacked[:, ND:D], psums[1][:])
                nc.scalar.dma_start(part_d.ap()[:, ND:D], stacked[:, ND:D])
            else:
                for h in range(NH):
                    nc.vector.tensor_copy(
                        stacked[:, h * ND:(h + 1) * ND], psums[h][:])
                    [nc.sync, nc.scalar][h % 2].dma_start(
                        part_d.ap()[:, h * ND:(h + 1) * ND],
                        stacked[:, h * ND:(h + 1) * ND])

    nc.compile()
    return nc


_CACHE = {}


def _get_nc():
    if "nc" not in _CACHE:
        _CACHE["nc"] = build()
    return _CACHE["nc"]


def _pack(arr, ncols):
    """[B_LOC, ncols] fp32 -> [128, NBLK, ncols] fp8, partition-major."""
    return np.ascontiguousarray(
        arr.reshape(NBLK, P, ncols).transpose(1, 0, 2)).astype(NP8)


def prep_in_maps(inputs, schedule=SCHEDULE):
    output = np.asarray(inputs["output"], dtype=np.float32)
    input = np.asarray(inputs["input"], dtype=np.float32)
    maps = []
    for c in range(N_CORES):
        packed = _pack(input[c * B_LOC:(c + 1) * B_LOC], D)
        m = {"output": _pack(output[c * B_LOC:(c + 1) * B_LOC], O)}
        blk0 = 0
        for i, ch in enumerate(schedule):
            m[f"in{i}"] = np.ascontiguousarray(
                packed[:, blk0:blk0 + ch, :]).reshape(P, ch * D)
            blk0 += ch
        maps.append(m)
    return maps


def kernel(output, input, weights):
    from concourse.bass_utils import run_bass_kernel_spmd

    weights = np.asarray(weights, dtype=np.float32)
    nc = _get_nc()
    in_maps = prep_in_maps({"output": output, "input": input})
    res = run_bass_kernel_spmd(nc, in_maps, list(range(N_CORES)))
    # host finish: sum 4 col-groups + 8 cores, then the tiny norm
    M = np.zeros((O, D), dtype=np.float64)
    for r in res.results:
        part = np.asarray(r["part"]).astype(np.float64)       # [128, D]
        M += part.reshape(NGRP, O, D).sum(axis=0)
    ss = (M * M).sum(axis=1)
    gw = np.sqrt(ss) * (weights.astype(np.float64) / B)
    return gw.astype(np.float32)


# revision 3
# speedup vs baseline: 1.0103x; 1.0100x over previous
"""Trainium2 Bass kernel for nn_GN_89266600280080.

Computes, for output[B,O], input[B,D], weights[O]:
    dl_dW = (1/B) * (output * weights)^T @ input        # [O, D]
    gw    = sqrt(sum(dl_dW^2, axis=1))                  # [O]

Strategy (8 NeuronCores, data-parallel over batch):
  - host: shard output/input on batch, pre-pack per-core slices into
    [128, n_blk, *] partition-major layout and quantize to fp8e4m3
    (norm over D=1024 averages quantization noise: ~3e-3 rel err vs
    2e-2 tolerance). Weight folding deferred to the final host scalar.
  - device: stacked partials = output_loc^T @ input_loc via fp8
    matmuls, 4-way column-tiled (tile_position=(0,32g)): 4 batch
    blocks stream concurrently through disjoint 32-col strips of the
    PE array.
  - input is staged per-chunk as its own contiguous DRAM tensor
    (descriptor p = base + p*chunk_bytes) so each chunk's 128 DMA
    descriptors sweep one contiguous DRAM span: measured ~390-410 GB/s
    on 8KB-line chunks vs ~320 for the partition-strided layout. All
    input DMA stays on the single sync HWDGE ring (two rings measured
    slower: interleaved descriptor streams thrash HBM).
  - schedule: tiny first chunk (quick first matmul under the cold-start
    DMA ramp), 8-block middle chunks (big descriptors), small tail
    chunks (short last-chunk drain).
  - w_out loads as TWO tiles (head=8 blocks first) so early matmuls
    only gate on the small head transfer, not the full 128KB.
  - tail: psum half 0 casts on DVE -> sync ring while half 1 casts on
    the Scalar (Act) engine -> scalar ring, overlapping the two casts
    and the two 128KB output DMAs.
  - each core emits its stacked [128, D] partial (bf16); host sums
    the 4 column groups + 8 cores and finishes the (tiny) norm.
"""

import sys
import numpy as np
import ml_dtypes

for _p in ("/opt/trn_rl_repo", "/root/.axon_site/_ro/trn_rl_repo"):
    if _p not in sys.path:
        sys.path.insert(0, _p)

B, O, D = 32768, 32, 1024
N_CORES = 8
B_LOC = B // N_CORES   # 4096
P = 128                # partitions
NBLK = B_LOC // P      # 32 batch blocks of 128 rows per core
NH = 2                 # D split into NH chunks of 512 for psum banks
ND = D // NH           # 512
NGRP = 4               # concurrent PE column groups
NP8 = ml_dtypes.float8_e4m3

SCHEDULE = (1, 2, 4, 8, 8, 4, 2, 2, 1)   # blocks per DMA chunk (sums to NBLK)
WHEAD = 8                                # w_out head blocks (first tile)


def build(n_cores=N_CORES, schedule=SCHEDULE, whead=WHEAD):
    """Build + compile the per-core Bass program. Returns the Bacc object."""
    import concourse.bacc as bacc
    import concourse.tile as tile
    import concourse.mybir as mybir

    f32 = mybir.dt.float32
    bf16 = mybir.dt.bfloat16
    f8 = mybir.dt.float8e4
    nsteps = NBLK // NGRP  # accumulation steps per column group
    assert sum(schedule) == NBLK

    nc = bacc.Bacc("TRN2", target_bir_lowering=False, debug=False,
                   num_devices=n_cores)

    # host pre-packed layouts: one contiguous DRAM tensor per input chunk
    ins = [nc.dram_tensor(f"in{c}", [P, ch * D], f8, kind="ExternalInput")
           for c, ch in enumerate(schedule)]
    wh_d = nc.dram_tensor("wouth", [P, whead * O], f8, kind="ExternalInput")
    wr_d = nc.dram_tensor("woutr", [P, (NBLK - whead) * O], f8,
                          kind="ExternalInput")
    part_d = nc.dram_tensor("part", [P, D], bf16, kind="ExternalOutput")

    with tile.TileContext(nc) as tc:
        with (
            tc.tile_pool(name="wout", bufs=2) as wout_pool,
            tc.tile_pool(name="rhs", bufs=len(schedule)) as rhs_pool,
            tc.tile_pool(name="ps", bufs=2, space="PSUM") as psum_pool,
            tc.tile_pool(name="misc", bufs=1) as misc,
        ):
            # stationary operand, split so early matmuls only wait on the head
            wout_h = wout_pool.tile([P, whead, O], f8, name="wout_h")
            wout_r = wout_pool.tile([P, NBLK - whead, O], f8, name="wout_r")
            nc.scalar.dma_start(
                wout_h[:].rearrange("p n d -> p (n d)"), wh_d.ap())
            nc.scalar.dma_start(
                wout_r[:].rearrange("p n d -> p (n d)"), wr_d.ap())

            # stacked partials: group g accumulates into partitions 32g..32g+31
            # separate psum tiles per D-half so each half's cast only waits
            # on that half's matmuls
            psums = [psum_pool.tile([P, ND], f32, name=f"psum{h}")
                     for h in range(NH)]
            blk0 = 0
            for c, ch in enumerate(schedule):
                rhs = rhs_pool.tile([P, ch, D], f8)
                nc.sync.dma_start(rhs[:].rearrange("p n d -> p (n d)"),
                                  ins[c].ap())
                for h in range(NH):
                    for j in range(ch):
                        blk = blk0 + j
                        g, t = blk % NGRP, blk // NGRP
                        w = (wout_h[:, blk, :] if blk < whead
                             else wout_r[:, blk - whead, :])
                        nc.tensor.matmul(
                            psums[h][O * g:O * (g + 1), :],
                            w,
                            rhs[:, j, h * ND:(h + 1) * ND],
                            start=(t == 0),
                            stop=(t == nsteps - 1),
                            tile_position=(0, O * g),
                        )
                blk0 += ch

            # tail: the two psum halves cast on different engines and ship
            # on different HWDGE rings so they overlap
            stacked = misc.tile([P, D], bf16)
            nc.vector.tensor_copy(stacked[:, 0:ND], psums[0][:])
            nc.sync.dma_start(part_d.ap()[:, 0:ND], stacked[:, 0:ND])
            nc.scalar.copy(stacked[:, ND:D], psums[1][:])
            nc.scalar.dma_start(part_d.ap()[:, ND:D], stacked[:, ND:D])

    nc.compile()
    return nc


_CACHE = {}


def _get_nc():
    if "nc" not in _CACHE:
        _CACHE["nc"] = build()
    return _CACHE["nc"]


def _pack(arr, ncols):
    """[B_LOC, ncols] fp32 -> [128, NBLK, ncols] fp8, partition-major."""
    return np.ascontiguousarray(
        arr.reshape(NBLK, P, ncols).transpose(1, 0, 2)).astype(NP8)


def prep_in_maps(inputs, schedule=SCHEDULE, whead=WHEAD):
    output = np.asarray(inputs["output"], dtype=np.float32)
    input = np.asarray(inputs["input"], dtype=np.float32)
    maps = []
    for c in range(N_CORES):
        packed = _pack(input[c * B_LOC:(c + 1) * B_LOC], D)
        wp = _pack(output[c * B_LOC:(c + 1) * B_LOC], O)
        m = {
            "wouth": np.ascontiguousarray(
                wp[:, 0:whead, :]).reshape(P, whead * O),
            "woutr": np.ascontiguousarray(
                wp[:, whead:, :]).reshape(P, (NBLK - whead) * O),
        }
        blk0 = 0
        for i, ch in enumerate(schedule):
            m[f"in{i}"] = np.ascontiguousarray(
                packed[:, blk0:blk0 + ch, :]).reshape(P, ch * D)
            blk0 += ch
        maps.append(m)
    return maps


def kernel(output, input, weights):
    from concourse.bass_utils import run_bass_kernel_spmd

    weights = np.asarray(weights, dtype=np.float32)
    nc = _get_nc()
    in_maps = prep_in_maps({"output": output, "input": input})
    res = run_bass_kernel_spmd(nc, in_maps, list(range(N_CORES)))
    # host finish: sum 4 col-groups + 8 cores, then the tiny norm
    M = np.zeros((O, D), dtype=np.float64)
    for r in res.results:
        part = np.asarray(r["part"]).astype(np.float64)       # [128, D]
        M += part.reshape(NGRP, O, D).sum(axis=0)
    ss = (M * M).sum(axis=1)
    gw = np.sqrt(ss) * (weights.astype(np.float64) / B)
    return gw.astype(np.float32)


# revision 4
# speedup vs baseline: 1.0233x; 1.0129x over previous
"""Trainium2 Bass kernel for nn_GN_89266600280080.

Computes, for output[B,O], input[B,D], weights[O]:
    dl_dW = (1/B) * (output * weights)^T @ input        # [O, D]
    gw    = sqrt(sum(dl_dW^2, axis=1))                  # [O]

Strategy (8 NeuronCores, data-parallel over batch):
  - host: shard output/input on batch, pre-pack per-core slices into
    [128, n_blk, *] partition-major layout and quantize to fp8e4m3
    (norm over D=1024 averages quantization noise: ~3e-3 rel err vs
    2e-2 tolerance). Weight folding deferred to the final host scalar.
  - device: stacked partials = output_loc^T @ input_loc via fp8
    matmuls, 4-way column-tiled (tile_position=(0,32g)): 4 batch
    blocks stream concurrently through disjoint 32-col strips of the
    PE array.
  - input is staged per-chunk as its own contiguous DRAM tensor
    (descriptor p = base + p*chunk_bytes) so each chunk's 128 DMA
    descriptors sweep one contiguous DRAM span: measured ~390-410 GB/s
    on 8KB-line chunks vs ~320 for the partition-strided layout. All
    input DMA stays on the single sync HWDGE ring (two rings measured
    slower: interleaved descriptor streams thrash HBM).
  - schedule: tiny first chunk (quick first matmul under the cold-start
    DMA ramp), 8-block middle chunks (big descriptors), small tail
    chunks (short last-chunk drain).
  - w_out loads as TWO tiles (head=8 blocks first) so early matmuls
    only gate on the small head transfer, not the full 128KB.
  - tail: psum half 0 casts on DVE -> sync ring while half 1 casts on
    the Scalar (Act) engine -> scalar ring, overlapping the two casts
    and the two 128KB output DMAs.
  - each core emits its stacked [128, D] partial (bf16); host sums
    the 4 column groups + 8 cores and finishes the (tiny) norm.
"""

import sys
import numpy as np
import ml_dtypes

for _p in ("/opt/trn_rl_repo", "/root/.axon_site/_ro/trn_rl_repo"):
    if _p not in sys.path:
        sys.path.insert(0, _p)

B, O, D = 32768, 32, 1024
N_CORES = 8
B_LOC = B // N_CORES   # 4096
P = 128                # partitions
NBLK = B_LOC // P      # 32 batch blocks of 128 rows per core
NH = 2                 # D split into NH chunks of 512 for psum banks
ND = D // NH           # 512
NGRP = 4               # concurrent PE column groups
NP8 = ml_dtypes.float8_e4m3

SCHEDULE = (1, 2, 8, 8, 8, 2, 2, 1)   # blocks per DMA chunk (sums to NBLK)
WHEAD = 8                                # w_out head blocks (first tile)


def build(n_cores=N_CORES, schedule=SCHEDULE, whead=WHEAD):
    """Build + compile the per-core Bass program. Returns the Bacc object."""
    import concourse.bacc as bacc
    import concourse.tile as tile
    import concourse.mybir as mybir

    f32 = mybir.dt.float32
    bf16 = mybir.dt.bfloat16
    f8 = mybir.dt.float8e4
    nsteps = NBLK // NGRP  # accumulation steps per column group
    assert sum(schedule) == NBLK

    nc = bacc.Bacc("TRN2", target_bir_lowering=False, debug=False,
                   num_devices=n_cores)

    # host pre-packed layouts: one contiguous DRAM tensor per input chunk
    ins = [nc.dram_tensor(f"in{c}", [P, ch * D], f8, kind="ExternalInput")
           for c, ch in enumerate(schedule)]
    wh_d = nc.dram_tensor("wouth", [P, whead * O], f8, kind="ExternalInput")
    wr_d = nc.dram_tensor("woutr", [P, (NBLK - whead) * O], f8,
                          kind="ExternalInput")
    part_d = nc.dram_tensor("part", [P, D], bf16, kind="ExternalOutput")

    with tile.TileContext(nc) as tc:
        with (
            tc.tile_pool(name="wout", bufs=2) as wout_pool,
            tc.tile_pool(name="rhs", bufs=len(schedule)) as rhs_pool,
            tc.tile_pool(name="ps", bufs=2, space="PSUM") as psum_pool,
            tc.tile_pool(name="misc", bufs=1) as misc,
        ):
            # stationary operand, split so early matmuls only wait on the head
            wout_h = wout_pool.tile([P, whead, O], f8, name="wout_h")
            wout_r = wout_pool.tile([P, NBLK - whead, O], f8, name="wout_r")
            nc.scalar.dma_start(
                wout_h[:].rearrange("p n d -> p (n d)"), wh_d.ap())
            nc.scalar.dma_start(
                wout_r[:].rearrange("p n d -> p (n d)"), wr_d.ap())

            # stacked partials: group g accumulates into partitions 32g..32g+31
            # separate psum tiles per D-half so each half's cast only waits
            # on that half's matmuls
            psums = [psum_pool.tile([P, ND], f32, name=f"psum{h}")
                     for h in range(NH)]
            blk0 = 0
            for c, ch in enumerate(schedule):
                rhs = rhs_pool.tile([P, ch, D], f8)
                nc.sync.dma_start(rhs[:].rearrange("p n d -> p (n d)"),
                                  ins[c].ap())
                for h in range(NH):
                    for j in range(ch):
                        blk = blk0 + j
                        g, t = blk % NGRP, blk // NGRP
                        w = (wout_h[:, blk, :] if blk < whead
                             else wout_r[:, blk - whead, :])
                        nc.tensor.matmul(
                            psums[h][O * g:O * (g + 1), :],
                            w,
                            rhs[:, j, h * ND:(h + 1) * ND],
                            start=(t == 0),
                            stop=(t == nsteps - 1),
                            tile_position=(0, O * g),
                        )
                blk0 += ch

            # tail: the two psum halves cast on different engines and ship
            # on different HWDGE rings so they overlap
            stacked = misc.tile([P, D], bf16)
            nc.vector.tensor_copy(stacked[:, 0:ND], psums[0][:])
            nc.sync.dma_start(part_d.ap()[:, 0:ND], stacked[:, 0:ND])
            nc.scalar.copy(stacked[:, ND:D], psums[1][:])
            nc.scalar.dma_start(part_d.ap()[:, ND:D], stacked[:, ND:D])

    nc.compile()
    return nc


_CACHE = {}


def _get_nc():
    if "nc" not in _CACHE:
        _CACHE["nc"] = build()
    return _CACHE["nc"]


def _pack(arr, ncols):
    """[B_LOC, ncols] fp32 -> [128, NBLK, ncols] fp8, partition-major."""
    return np.ascontiguousarray(
        arr.reshape(NBLK, P, ncols).transpose(1, 0, 2)).astype(NP8)


def prep_in_maps(inputs, schedule=SCHEDULE, whead=WHEAD):
    output = np.asarray(inputs["output"], dtype=np.float32)
    input = np.asarray(inputs["input"], dtype=np.float32)
    maps = []
    for c in range(N_CORES):
        packed = _pack(input[c * B_LOC:(c + 1) * B_LOC], D)
        wp = _pack(output[c * B_LOC:(c + 1) * B_LOC], O)
        m = {
            "wouth": np.ascontiguousarray(
                wp[:, 0:whead, :]).reshape(P, whead * O),
            "woutr": np.ascontiguousarray(
                wp[:, whead:, :]).reshape(P, (NBLK - whead) * O),
        }
        blk0 = 0
        for i, ch in enumerate(schedule):
            m[f"in{i}"] = np.ascontiguousarray(
                packed[:, blk0:blk0 + ch, :]).reshape(P, ch * D)
            blk0 += ch
        maps.append(m)
    return maps


def kernel(output, input, weights):
    from concourse.bass_utils import run_bass_kernel_spmd

    weights = np.asarray(weights, dtype=np.float32)
    nc = _get_nc()
    in_maps = prep_in_maps({"output": output, "input": input})
    res = run_bass_kernel_spmd(nc, in_maps, list(range(N_CORES)))
    # host finish: sum 4 col-groups + 8 cores, then the tiny norm
    M = np.zeros((O, D), dtype=np.float64)
    for r in res.results:
        part = np.asarray(r["part"]).astype(np.float64)       # [128, D]
        M += part.reshape(NGRP, O, D).sum(axis=0)
    ss = (M * M).sum(axis=1)
    gw = np.sqrt(ss) * (weights.astype(np.float64) / B)
    return gw.astype(np.float32)
